# revision 1
# baseline (speedup 1.0000x reference)
"""PiCANet-G attention module as a Trainium2 Bass/Tile kernel.

Pure data-parallel over batch: 64 samples -> 8 cores x 8 samples.

Per core, three phases (all SBUF-resident, bf16 matmuls, fp32 cell state):
  P1: vertical bi-LSTM over W (batch = 8*28 (b, h) rows, 28 steps, 2 dirs)
  P2: horizontal bi-LSTM over H (batch = 8*28 (b, w) rows)
  P3: fc -> softmax(100) -> per-sample einsum with the dilated 10x10 patch

Recurrence layout: gates G[1024, 224] with the gate dim on partitions
(8 m-tiles packed pairwise into 4 PSUM banks); hidden state h[256, 224]
is produced directly in the layout the next step's matmul consumes (rhs
with K on partitions) so there are no per-step transposes. Weights are
pre-transposed/permuted on the host (not part of device exec time).
"""

import numpy as np
import ml_dtypes
from contextlib import ExitStack

import concourse.bacc as bacc
import concourse.mybir as mybir
import concourse.tile as tile
from concourse.masks import make_identity
from concourse.bass_utils import run_bass_kernel_spmd

# problem shapes (hardcoded per contract)
B, C, H, W = 64, 512, 28, 28
HID = 256
N_CORES = 8
BL = B // N_CORES        # samples per core
NB = BL * H              # 224 rows per LSTM step
T = 28                   # steps per LSTM
PLOC = BL * H * W        # 6272 positions per core

BF16 = mybir.dt.bfloat16
F32 = mybir.dt.float32
AF = mybir.ActivationFunctionType

# torch gate order [i f g o] -> device order [i f o g] (sigmoids first)
_PERM = np.concatenate([np.arange(0, 512), np.arange(768, 1024), np.arange(512, 768)])
_GATE_FUNC = [AF.Sigmoid, AF.Sigmoid, AF.Sigmoid, AF.Tanh]

_LSTMS = ["vf", "vb", "hf", "hb"]


def _emit_lstm_step(nc, gpool, scr, wih_sb, whh_sb, bias_sb, src_rhs, dst_slab,
                    c_ap, dir_i, t, name, has_bias=True):
    """One LSTM step for one direction. src_rhs(kk, pos) -> [128, 224] AP."""
    pos = t if dir_i == 0 else T - 1 - t
    prev = pos - 1 if dir_i == 0 else pos + 1
    gates = []
    for gate in range(4):
        gt = gpool.tile([128, 512], F32, tag=f"g{gate}", name=f"g_{name}_{t}_{gate}")
        for half in range(2):
            m = gate * 2 + half
            out_ap = gt[:, half * 256: half * 256 + 224]
            for kk in range(4):
                nc.tensor.matmul(
                    out_ap,
                    lhsT=wih_sb[:, kk, m * 128:(m + 1) * 128],
                    rhs=src_rhs(kk, pos),
                    start=(half == 0 and kk == 0),
                    stop=(t == 0 and half == 1 and kk == 3),
                )
            if t > 0:
                for kk in range(2):
                    nc.tensor.matmul(
                        out_ap,
                        lhsT=whh_sb[:, kk, m * 128:(m + 1) * 128],
                        rhs=dst_slab[:, dir_i * 2 + kk, prev * 224:(prev + 1) * 224],
                        start=False,
                        stop=(half == 1 and kk == 1),
                    )
        gv = gt.rearrange("p (two x) -> p two x", two=2)[:, :, 0:224]
        if gate == 3:
            # tanh(g) to SBUF so the i*g product has only one PSUM operand
            tg = scr.tile([128, 2, 224], F32, tag="tg", bufs=3,
                          name=f"tg_{name}_{t}")
            if has_bias:
                for half in range(2):
                    m = gate * 2 + half
                    nc.scalar.activation(tg[:, half, :], gv[:, half, :],
                                         _GATE_FUNC[gate],
                                         bias=bias_sb[:, m:m + 1])
            else:
                nc.scalar.activation(tg, gv, _GATE_FUNC[gate])
            gates.append(tg)
        else:
            if has_bias:
                for half in range(2):
                    m = gate * 2 + half
                    # fused bias + nonlinearity, in place in PSUM
                    nc.scalar.activation(gv[:, half, :], gv[:, half, :],
                                         _GATE_FUNC[gate],
                                         bias=bias_sb[:, m:m + 1])
            else:
                # biases all zero: one activation over both halves (gap skipped)
                nc.scalar.activation(gv, gv, _GATE_FUNC[gate])
            gates.append(gv)
    g_i, g_f, g_o, g_g = gates

    if t == 0:
        nc.vector.tensor_mul(c_ap, g_i, g_g)
    else:
        t1 = scr.tile([128, 2, 224], F32, tag="t1", bufs=3, name=f"t1_{name}_{t}")
        nc.vector.tensor_mul(t1, g_i, g_g)
        nc.vector.tensor_mul(c_ap, g_f, c_ap)
        nc.vector.tensor_add(c_ap, c_ap, t1)
    th = scr.tile([128, 2, 224], F32, tag="th", bufs=3, name=f"th_{name}_{t}")
    nc.scalar.activation(th, c_ap, AF.Tanh)
    # h -> bf16 slab, both hidden halves in one strided write
    h_ap = dst_slab[:, dir_i * 2:dir_i * 2 + 2, pos * 224:(pos + 1) * 224]
    nc.vector.tensor_mul(h_ap, g_o, th)


def _build(reps=1, debug=False, has_bias=True):
    nc = bacc.Bacc(None, target_bir_lowering=False)

    xT_d = nc.dram_tensor("xT", [C, PLOC], BF16, kind="ExternalInput")
    w_d = {}
    for L in _LSTMS:
        w_d[L + "_wih"] = nc.dram_tensor(L + "_wih", [512, 1024], BF16, kind="ExternalInput")
        w_d[L + "_whh"] = nc.dram_tensor(L + "_whh", [256, 1024], BF16, kind="ExternalInput")
        w_d[L + "_bias"] = nc.dram_tensor(L + "_bias", [128, 8], F32, kind="ExternalInput")
    fcw_d = nc.dram_tensor("fcw", [512, 100], BF16, kind="ExternalInput")
    fcb_d = nc.dram_tensor("fcb", [1, 100], BF16, kind="ExternalInput")
    patchT_d = nc.dram_tensor("patchT", [BL, 100, 512], BF16, kind="ExternalInput")
    out_d = nc.dram_tensor("out", [BL, C, H * W], F32, kind="ExternalOutput")
    if debug:
        dbg_hv = nc.dram_tensor("dbg_hv", [128, 4, PLOC], BF16, kind="ExternalOutput")
        dbg_hh = nc.dram_tensor("dbg_hh", [128, 4, PLOC], BF16, kind="ExternalOutput")
        dbg_kt = nc.dram_tensor("dbg_kt", [100, PLOC], BF16, kind="ExternalOutput")

    with tile.TileContext(nc) as tc, ExitStack() as ctx:
        wpool = ctx.enter_context(tc.tile_pool(name="wpool", bufs=1))
        bigA = ctx.enter_context(tc.tile_pool(name="bigA", bufs=1))
        bigB = ctx.enter_context(tc.tile_pool(name="bigB", bufs=1))
        state = ctx.enter_context(tc.tile_pool(name="state", bufs=1))
        scr = ctx.enter_context(tc.tile_pool(name="scr", bufs=3))

        # --- load weights; both stage-1 dirs first (step 0 needs them) ---
        wih_sb, whh_sb, bias_sb = {}, {}, {}
        for L in _LSTMS:
            wih_sb[L] = wpool.tile([128, 4, 1024], BF16, name=f"wih_{L}")
            whh_sb[L] = wpool.tile([128, 2, 1024], BF16, name=f"whh_{L}")
            bias_sb[L] = wpool.tile([128, 8], F32, name=f"bias_{L}")
        for L in ["vf", "vb"]:
            nc.sync.dma_start(out=wih_sb[L],
                              in_=w_d[L + "_wih"].rearrange("(kt p) m -> p kt m", kt=4))
        for L in ["vf", "vb"]:
            nc.scalar.dma_start(out=whh_sb[L],
                                in_=w_d[L + "_whh"].rearrange("(kt p) m -> p kt m", kt=2))
            if has_bias:
                nc.scalar.dma_start(out=bias_sb[L], in_=w_d[L + "_bias"][:, :])
        for L in ["hf", "hb"]:
            nc.sync.dma_start(out=wih_sb[L],
                              in_=w_d[L + "_wih"].rearrange("(kt p) m -> p kt m", kt=4))
            nc.sync.dma_start(out=whh_sb[L],
                              in_=w_d[L + "_whh"].rearrange("(kt p) m -> p kt m", kt=2))
            if has_bias:
                nc.sync.dma_start(out=bias_sb[L], in_=w_d[L + "_bias"][:, :])
        fcw_sb = wpool.tile([128, 4, 100], BF16, name="fcw_sb")
        nc.sync.dma_start(out=fcw_sb, in_=fcd_rearr(fcw_d))
        if has_bias:
            fcb_sb = wpool.tile([1, 100], BF16, name="fcb_sb")
            nc.sync.dma_start(out=fcb_sb, in_=fcb_d[:, :])
            ones112 = wpool.tile([1, 112], BF16, name="ones112")
            nc.vector.memset(ones112, 1.0)
        else:
            fcb_sb = ones112 = None
        patchT_sb = wpool.tile([100, BL, 512], BF16, name="patchT_sb")
        nc.sync.dma_start(out=patchT_sb, in_=patchT_d.rearrange("b k c -> k b c"))
        ident = wpool.tile([112, 112], F32, name="ident")
        make_identity(nc, ident)

        for rep in range(reps):
            sfx = f"r{rep}"
            # --- P1: vertical bi-LSTM ---
            xT = bigA.tile([128, 4, PLOC], BF16, tag="bigA", name=f"xT_{sfx}")
            xsrc = xT_d.rearrange("(kt p) f -> p kt f", kt=4)
            # stream in the order both directions consume: edges first
            wblocks = [(0, 3), (25, 28), (3, 8), (20, 25), (8, 14), (14, 20)]
            for lo, hi in wblocks:
                for kk in range(4):
                    nc.scalar.dma_start(out=xT[:, kk, lo * 224:hi * 224],
                                        in_=xsrc[:, kk, lo * 224:hi * 224])
            Hv = bigB.tile([128, 4, PLOC], BF16, tag="bigB", name=f"Hv_{sfx}")

            def rhs1(kk, pos, _xT=xT):
                # xT free layout is (w, b, h): one contiguous slice per step
                return _xT[:, kk, pos * 224:(pos + 1) * 224]

            with tc.tile_pool(name="gates1", bufs=2, space="PSUM") as gpool:
                cs = [state.tile([128, 2, 224], F32, tag=f"c1_{d}",
                                 name=f"c1_{d}_{sfx}") for d in range(2)]
                for t in range(T):
                    for d, L in enumerate(["vf", "vb"]):
                        _emit_lstm_step(nc, gpool, scr, wih_sb[L], whh_sb[L],
                                        bias_sb[L], rhs1, Hv, cs[d], d, t,
                                        f"1{L}{sfx}", has_bias=has_bias)

            # --- P2: horizontal bi-LSTM ---
            Hh = bigA.tile([128, 4, PLOC], BF16, tag="bigA", name=f"Hh_{sfx}")

            def rhs2(kk, pos, _Hv=Hv):
                a = _Hv[:, kk, :].rearrange("p (w b h) -> p b w h", w=W, b=BL)
                return a[:, :, :, pos]

            with tc.tile_pool(name="gates2", bufs=2, space="PSUM") as gpool:
                cs = [state.tile([128, 2, 224], F32, tag=f"c2_{d}",
                                 name=f"c2_{d}_{sfx}") for d in range(2)]
                for t in range(T):
                    for d, L in enumerate(["hf", "hb"]):
                        _emit_lstm_step(nc, gpool, scr, wih_sb[L], whh_sb[L],
                                        bias_sb[L], rhs2, Hh, cs[d], d, t,
                                        f"2{L}{sfx}", has_bias=has_bias)

            # --- P3: fc + softmax + transpose + einsum ---
            KT = bigB.tile([100, PLOC], BF16, tag="bigB", name=f"KT_{sfx}")
            with tc.tile_pool(name="p3ps", bufs=2, space="PSUM") as pps:
                ci = 0
                for half in range(2):
                    # fc + softmax + transpose for samples b in 4*half..4*half+3
                    for hr in range(H):
                        off = hr * 224 + half * 112
                        Lp = pps.tile([112, 100], F32, tag="L", name=f"L_{hr}_{half}_{sfx}")
                        for kk in range(4):
                            lhsT = Hh[:, kk, off:off + 112]
                            nc.tensor.matmul(Lp, lhsT=lhsT, rhs=fcw_sb[:, kk, :],
                                             start=(kk == 0),
                                             stop=(not has_bias and kk == 3))
                        if has_bias:
                            nc.tensor.matmul(Lp, lhsT=ones112, rhs=fcb_sb,
                                             start=False, stop=True)
                        E = scr.tile([112, 100], F32, tag="E", bufs=3,
                                     name=f"E_{hr}_{half}_{sfx}")
                        Zs = scr.tile([112, 1], F32, tag="Z", bufs=3,
                                      name=f"Z_{hr}_{half}_{sfx}")
                        nc.scalar.activation(E, Lp, AF.Exp, accum_out=Zs)
                        rz = scr.tile([112, 1], F32, tag="rz", bufs=3,
                                      name=f"rz_{hr}_{half}_{sfx}")
                        nc.vector.reciprocal(rz, Zs)
                        Ka = scr.tile([112, 100], F32, tag="Ka", bufs=3,
                                      name=f"Ka_{hr}_{half}_{sfx}")
                        nc.vector.tensor_scalar_mul(Ka, E, rz)
                        KTp = pps.tile([100, 112], F32, tag="KTp",
                                       name=f"KTp_{hr}_{half}_{sfx}")
                        nc.tensor.transpose(KTp, Ka, ident)
                        # KT columns p = b*784 + hr*28 + w for these positions
                        dst = KT.rearrange("k (b hw) -> k b hw", b=BL)[
                            :, half * 4:(half + 1) * 4, hr * 28:(hr + 1) * 28]
                        if ci % 2 == 0:
                            nc.vector.tensor_copy(dst, KTp)
                        else:
                            nc.scalar.copy(dst, KTp)
                        ci += 1
                    # einsum for this half's samples (overlaps the other half's fc)
                    for b_i in range(half * 4, (half + 1) * 4):
                        for ct in range(4):
                            lhsT = patchT_sb[:, b_i, ct * 128:(ct + 1) * 128]
                            # [128, 1024] = 2 PSUM banks; each matmul output
                            # must stay inside one bank, so halves go at 0/512
                            Op = pps.tile([128, 2, 512], F32, tag="O", bufs=2,
                                          name=f"O_{b_i}_{ct}_{sfx}")
                            for j2 in range(2):
                                nc.tensor.matmul(
                                    Op[:, j2, 0:392], lhsT=lhsT,
                                    rhs=KT[:, b_i * 784 + j2 * 392:
                                           b_i * 784 + (j2 + 1) * 392],
                                    start=True, stop=True)
                            ob = scr.tile([128, 2, 392], F32, tag="ob", bufs=3,
                                          name=f"ob_{b_i}_{ct}_{sfx}")
                            if ct % 2 == 0:
                                nc.vector.tensor_copy(ob, Op[:, :, 0:392])
                            else:
                                nc.scalar.copy(ob, Op[:, :, 0:392])
                            eng = nc.sync if ct % 2 == 0 else nc.scalar
                            eng.dma_start(
                                out=out_d[b_i, ct * 128:(ct + 1) * 128, :],
                                in_=ob)
            if debug and rep == reps - 1:
                nc.sync.dma_start(out=dbg_hv[:, :, :], in_=Hv)
                nc.sync.dma_start(out=dbg_hh[:, :, :], in_=Hh)
                nc.sync.dma_start(out=dbg_kt[:, :], in_=KT)

    nc.compile()
    return nc


def fcd_rearr(fcw_d):
    return fcw_d.rearrange("(kt p) n -> p kt n", kt=4)


_NC_CACHE = {}


def _get_nc(reps=1, debug=False, has_bias=True):
    key = (reps, debug, has_bias)
    if key not in _NC_CACHE:
        _NC_CACHE[key] = _build(reps=reps, debug=debug, has_bias=has_bias)
    return _NC_CACHE[key]


def _prep_core_inputs(x, weights_np):
    """Host-side marshalling for one core. x: [BL, C, H, W] f32."""
    bf = ml_dtypes.bfloat16
    m = {}
    m["xT"] = np.ascontiguousarray(
        x.transpose(1, 3, 0, 2).reshape(C, PLOC)).astype(bf)
    m["patchT"] = np.ascontiguousarray(
        x[:, :, ::3, ::3].reshape(BL, C, 100).transpose(0, 2, 1)).astype(bf)
    m.update(weights_np)
    return m


def _prep_weights(inputs):
    bf = ml_dtypes.bfloat16
    w = {}
    for L in _LSTMS:
        wih = np.asarray(inputs[L + "_Wih"], np.float32)
        whh = np.asarray(inputs[L + "_Whh"], np.float32)
        bih = np.asarray(inputs[L + "_bih"], np.float32)
        bhh = np.asarray(inputs[L + "_bhh"], np.float32)
        w[L + "_wih"] = np.ascontiguousarray(wih[_PERM].T).astype(bf)
        w[L + "_whh"] = np.ascontiguousarray(whh[_PERM].T).astype(bf)
        w[L + "_bias"] = np.ascontiguousarray(
            (bih + bhh)[_PERM].reshape(8, 128).T).astype(np.float32)
    w["fcw"] = np.asarray(inputs["fc_W"], np.float32).astype(bf)
    w["fcb"] = np.asarray(inputs["fc_b"], np.float32).reshape(1, 100).astype(bf)
    return w


def run_cores(inputs, reps=1, debug=False):
    x = np.asarray(inputs["x"], np.float32)
    wnp = _prep_weights(inputs)
    has_bias = any(np.any(wnp[L + "_bias"]) for L in _LSTMS)
    nc = _get_nc(reps=reps, debug=debug, has_bias=has_bias)
    in_maps = [
        _prep_core_inputs(x[ci * BL:(ci + 1) * BL], wnp) for ci in range(N_CORES)
    ]
    res = run_bass_kernel_spmd(nc, in_maps, list(range(N_CORES)))
    return res


def kernel(**inputs) -> np.ndarray:
    res = run_cores(inputs)
    out = np.concatenate(
        [res.results[ci]["out"].reshape(BL, C, H, W) for ci in range(N_CORES)],
        axis=0)
    return out.astype(np.float32)



# revision 7
# speedup vs baseline: 1.2945x; 1.2945x over previous
"""PiCANet-G attention module as a Trainium2 Bass/Tile kernel.

Pure data-parallel over batch: 64 samples -> 8 cores x 8 samples.

Per core, three phases (all SBUF-resident, bf16 matmuls, fp32 cell state):
  P1: vertical bi-LSTM over W (batch = 8*28 (b, h) rows, 28 steps, 2 dirs)
  P2: horizontal bi-LSTM over H (batch = 8*28 (b, w) rows)
  P3: fc -> softmax(100) -> per-sample einsum with the dilated 10x10 patch

Recurrence layout: gates G[1024, 224] with the gate dim on partitions
(8 m-tiles packed pairwise into 4 PSUM banks); hidden state h[256, 224]
is produced directly in the layout the next step's matmul consumes (rhs
with K on partitions) so there are no per-step transposes. Weights are
pre-transposed/permuted on the host (not part of device exec time).
"""

import numpy as np
import ml_dtypes
from contextlib import ExitStack

import concourse.bacc as bacc
import concourse.mybir as mybir
import concourse.tile as tile
from concourse.bass_utils import run_bass_kernel_spmd

# problem shapes (hardcoded per contract)
B, C, H, W = 64, 512, 28, 28
HID = 256
N_CORES = 8
BL = B // N_CORES        # samples per core
NB = BL * H              # 224 rows per LSTM step
T = 28                   # steps per LSTM
PLOC = BL * H * W        # 6272 positions per core

BF16 = mybir.dt.bfloat16
F32 = mybir.dt.float32
AF = mybir.ActivationFunctionType

# torch gate order [i f g o] -> device order [i f o g] (sigmoids first)
_PERM = np.concatenate([np.arange(0, 512), np.arange(768, 1024), np.arange(512, 768)])
_GATE_FUNC = [AF.Sigmoid, AF.Sigmoid, AF.Sigmoid, AF.Tanh]

_LSTMS = ["vf", "vb", "hf", "hb"]


def _emit_lstm_step(nc, gpool, scr, wih_sb, whh_sb, bias_sb, src_rhs, dst_slab,
                    c_ap, dir_i, t, name, has_bias=True):
    """One LSTM step for one direction. src_rhs(kk, pos) -> [128, 224] AP."""
    pos = t if dir_i == 0 else T - 1 - t
    prev = pos - 1 if dir_i == 0 else pos + 1
    gates = []
    for gate in range(4):
        gt = gpool.tile([128, 512], F32, tag=f"g{gate}", name=f"g_{name}_{t}_{gate}")
        for half in range(2):
            m = gate * 2 + half
            out_ap = gt[:, half * 256: half * 256 + 224]
            for kk in range(4):
                nc.tensor.matmul(
                    out_ap,
                    lhsT=wih_sb[:, kk, m * 128:(m + 1) * 128],
                    rhs=src_rhs(kk, pos),
                    start=(half == 0 and kk == 0),
                    stop=(t == 0 and half == 1 and kk == 3),
                )
            if t > 0:
                for kk in range(2):
                    nc.tensor.matmul(
                        out_ap,
                        lhsT=whh_sb[:, kk, m * 128:(m + 1) * 128],
                        rhs=dst_slab[:, dir_i * 2 + kk, prev * 224:(prev + 1) * 224],
                        start=False,
                        stop=(half == 1 and kk == 1),
                    )
        gv = gt.rearrange("p (two x) -> p two x", two=2)[:, :, 0:224]
        if gate == 3:
            # tanh(g) to SBUF so the i*g product has only one PSUM operand
            tg = scr.tile([128, 2, 224], F32, tag="tg", bufs=3,
                          name=f"tg_{name}_{t}")
            if has_bias:
                for half in range(2):
                    m = gate * 2 + half
                    nc.scalar.activation(tg[:, half, :], gv[:, half, :],
                                         _GATE_FUNC[gate],
                                         bias=bias_sb[:, m:m + 1])
            else:
                nc.scalar.activation(tg, gv, _GATE_FUNC[gate])
            gates.append(tg)
        else:
            if has_bias:
                for half in range(2):
                    m = gate * 2 + half
                    # fused bias + nonlinearity, in place in PSUM
                    nc.scalar.activation(gv[:, half, :], gv[:, half, :],
                                         _GATE_FUNC[gate],
                                         bias=bias_sb[:, m:m + 1])
            else:
                # biases all zero: one activation over both halves (gap skipped)
                nc.scalar.activation(gv, gv, _GATE_FUNC[gate])
            gates.append(gv)
    g_i, g_f, g_o, g_g = gates

    if t == 0:
        nc.vector.tensor_mul(c_ap, g_i, g_g)
    else:
        t1 = scr.tile([128, 2, 224], F32, tag="t1", bufs=3, name=f"t1_{name}_{t}")
        nc.vector.tensor_mul(t1, g_i, g_g)
        nc.vector.tensor_mul(c_ap, g_f, c_ap)
        nc.vector.tensor_add(c_ap, c_ap, t1)
    th = scr.tile([128, 2, 224], F32, tag="th", bufs=3, name=f"th_{name}_{t}")
    nc.scalar.activation(th, c_ap, AF.Tanh)
    # h -> bf16 slab, both hidden halves in one strided write
    h_ap = dst_slab[:, dir_i * 2:dir_i * 2 + 2, pos * 224:(pos + 1) * 224]
    nc.vector.tensor_mul(h_ap, g_o, th)


def _build(reps=1, debug=False, has_bias=True):
    nc = bacc.Bacc(None, target_bir_lowering=False)

    xT_d = nc.dram_tensor("xT", [C, PLOC], BF16, kind="ExternalInput")
    w_d = {}
    for L in _LSTMS:
        w_d[L + "_wih"] = nc.dram_tensor(L + "_wih", [512, 1024], BF16, kind="ExternalInput")
        w_d[L + "_whh"] = nc.dram_tensor(L + "_whh", [256, 1024], BF16, kind="ExternalInput")
        w_d[L + "_bias"] = nc.dram_tensor(L + "_bias", [128, 8], F32, kind="ExternalInput")
    fcw_d = nc.dram_tensor("fcw", [512, 100], BF16, kind="ExternalInput")
    fcb_d = nc.dram_tensor("fcb", [100, 1], F32, kind="ExternalInput")
    patchT_d = nc.dram_tensor("patchT", [BL, 100, 512], BF16, kind="ExternalInput")
    out_d = nc.dram_tensor("out", [BL, C, H * W], F32, kind="ExternalOutput")
    if debug:
        dbg_hv = nc.dram_tensor("dbg_hv", [128, 4, PLOC], BF16, kind="ExternalOutput")
        dbg_hh = nc.dram_tensor("dbg_hh", [128, 4, PLOC], BF16, kind="ExternalOutput")
        dbg_kt = nc.dram_tensor("dbg_kt", [100, PLOC], BF16, kind="ExternalOutput")

    with tile.TileContext(nc) as tc, ExitStack() as ctx:
        wpool = ctx.enter_context(tc.tile_pool(name="wpool", bufs=1))
        bigA = ctx.enter_context(tc.tile_pool(name="bigA", bufs=1))
        bigB = ctx.enter_context(tc.tile_pool(name="bigB", bufs=1))
        state = ctx.enter_context(tc.tile_pool(name="state", bufs=1))
        scr = ctx.enter_context(tc.tile_pool(name="scr", bufs=3))

        # --- load weights; both stage-1 dirs first (step 0 needs them) ---
        wih_sb, whh_sb, bias_sb = {}, {}, {}
        for L in _LSTMS:
            wih_sb[L] = wpool.tile([128, 4, 1024], BF16, name=f"wih_{L}")
            whh_sb[L] = wpool.tile([128, 2, 1024], BF16, name=f"whh_{L}")
            bias_sb[L] = wpool.tile([128, 8], F32, name=f"bias_{L}")
        for L in ["vf", "vb"]:
            nc.sync.dma_start(out=wih_sb[L],
                              in_=w_d[L + "_wih"].rearrange("(kt p) m -> p kt m", kt=4))
        for L in ["vf", "vb"]:
            nc.scalar.dma_start(out=whh_sb[L],
                                in_=w_d[L + "_whh"].rearrange("(kt p) m -> p kt m", kt=2))
            if has_bias:
                nc.scalar.dma_start(out=bias_sb[L], in_=w_d[L + "_bias"][:, :])
        for L in ["hf", "hb"]:
            nc.sync.dma_start(out=wih_sb[L],
                              in_=w_d[L + "_wih"].rearrange("(kt p) m -> p kt m", kt=4))
            nc.sync.dma_start(out=whh_sb[L],
                              in_=w_d[L + "_whh"].rearrange("(kt p) m -> p kt m", kt=2))
            if has_bias:
                nc.sync.dma_start(out=bias_sb[L], in_=w_d[L + "_bias"][:, :])
        fcw_sb = wpool.tile([128, 4, 100], BF16, name="fcw_sb")
        nc.sync.dma_start(out=fcw_sb, in_=fcd_rearr(fcw_d))
        if has_bias:
            fcb_sb = wpool.tile([100, 1], F32, name="fcb_sb")
            nc.sync.dma_start(out=fcb_sb, in_=fcb_d[:, :])
        else:
            fcb_sb = None
        ones100 = wpool.tile([100, 1], BF16, name="ones100")
        nc.vector.memset(ones100, 1.0)
        patchT_sb = wpool.tile([100, BL, 512], BF16, name="patchT_sb")
        nc.sync.dma_start(out=patchT_sb, in_=patchT_d.rearrange("b k c -> k b c"))

        for rep in range(reps):
            sfx = f"r{rep}"
            # --- P1: vertical bi-LSTM ---
            xT = bigA.tile([128, 4, PLOC], BF16, tag="bigA", name=f"xT_{sfx}")
            xsrc = xT_d.rearrange("(kt p) f -> p kt f", kt=4)
            # stream in the order both directions consume: edges first
            wblocks = [(0, 3), (25, 28), (3, 8), (20, 25), (8, 14), (14, 20)]
            for lo, hi in wblocks:
                for kk in range(4):
                    nc.scalar.dma_start(out=xT[:, kk, lo * 224:hi * 224],
                                        in_=xsrc[:, kk, lo * 224:hi * 224])
            Hv = bigB.tile([128, 4, PLOC], BF16, tag="bigB", name=f"Hv_{sfx}")

            def rhs1(kk, pos, _xT=xT):
                # xT free layout is (w, b, h): one contiguous slice per step
                return _xT[:, kk, pos * 224:(pos + 1) * 224]

            with tc.tile_pool(name="gates1", bufs=2, space="PSUM") as gpool:
                cs = [state.tile([128, 2, 224], F32, tag=f"c1_{d}",
                                 name=f"c1_{d}_{sfx}") for d in range(2)]
                for t in range(T):
                    for d, L in enumerate(["vf", "vb"]):
                        _emit_lstm_step(nc, gpool, scr, wih_sb[L], whh_sb[L],
                                        bias_sb[L], rhs1, Hv, cs[d], d, t,
                                        f"1{L}{sfx}", has_bias=has_bias)

            # --- P2: horizontal bi-LSTM ---
            Hh = bigA.tile([128, 4, PLOC], BF16, tag="bigA", name=f"Hh_{sfx}")

            def rhs2(kk, pos, _Hv=Hv):
                a = _Hv[:, kk, :].rearrange("p (w b h) -> p b w h", w=W, b=BL)
                return a[:, :, :, pos]

            with tc.tile_pool(name="gates2", bufs=2, space="PSUM") as gpool:
                cs = [state.tile([128, 2, 224], F32, tag=f"c2_{d}",
                                 name=f"c2_{d}_{sfx}") for d in range(2)]
                for t in range(T):
                    for d, L in enumerate(["hf", "hb"]):
                        _emit_lstm_step(nc, gpool, scr, wih_sb[L], whh_sb[L],
                                        bias_sb[L], rhs2, Hh, cs[d], d, t,
                                        f"2{L}{sfx}", has_bias=has_bias)

            # --- P3: transposed fc (100 on partitions) + softmax + einsum ---
            # E[k, p] for positions p = (h, b, w); normalized in place.
            E = bigB.tile([100, PLOC], BF16, tag="bigB", name=f"E_{sfx}")
            NCH = 16
            CH = PLOC // NCH  # 392
            with tc.tile_pool(name="p3ps", bufs=2, space="PSUM") as pps:
                for c in range(NCH):
                    sl = slice(c * CH, (c + 1) * CH)
                    Lp = pps.tile([100, CH], F32, tag="L", name=f"L_{c}_{sfx}")
                    for kk in range(4):
                        nc.tensor.matmul(Lp, lhsT=fcw_sb[:, kk, :],
                                         rhs=Hh[:, kk, sl],
                                         start=(kk == 0), stop=(kk == 3))
                    # exp(L + fc_b): bias is per-partition in this layout
                    if has_bias:
                        nc.scalar.activation(E[:, sl], Lp, AF.Exp, bias=fcb_sb)
                    else:
                        nc.scalar.activation(E[:, sl], Lp, AF.Exp)
                    Zp = pps.tile([1, CH], F32, tag="Z", name=f"Z_{c}_{sfx}")
                    nc.tensor.matmul(Zp, lhsT=ones100, rhs=E[:, sl],
                                     start=True, stop=True)
                    rz = scr.tile([1, CH], BF16, tag="rz", bufs=3,
                                  name=f"rz_{c}_{sfx}")
                    with nc.allow_low_precision(reason="softmax 1/Z in bf16"):
                        nc.vector.reciprocal(rz, Zp)
                    ib = scr.tile([100, CH], BF16, tag="ib", bufs=3,
                                  name=f"ib_{c}_{sfx}")
                    nc.gpsimd.partition_broadcast(ib, rz)
                    nc.vector.tensor_mul(E[:, sl], E[:, sl], ib)
                # einsum: out[c,(h,w)] = sum_k patch[c,k] En[k,(h,w)] per sample
                Ev = E.rearrange("k (h b w) -> k h b w", h=H, b=BL)
                for b_i in range(BL):
                    for ct in range(4):
                        lhsT = patchT_sb[:, b_i, ct * 128:(ct + 1) * 128]
                        # [128, 1024] = 2 PSUM banks; each matmul output
                        # must stay inside one bank, so halves go at 0/512
                        Op = pps.tile([128, 2, 512], F32, tag="O", bufs=2,
                                      name=f"O_{b_i}_{ct}_{sfx}")
                        for j2 in range(2):
                            nc.tensor.matmul(
                                Op[:, j2, 0:392], lhsT=lhsT,
                                rhs=Ev[:, j2 * 14:(j2 + 1) * 14, b_i, :],
                                start=True, stop=True)
                        ob = scr.tile([128, 2, 392], F32, tag="ob", bufs=3,
                                      name=f"ob_{b_i}_{ct}_{sfx}")
                        if ct % 2 == 0:
                            nc.vector.tensor_copy(ob, Op[:, :, 0:392])
                        else:
                            nc.scalar.copy(ob, Op[:, :, 0:392])
                        eng = nc.sync if ct % 2 == 0 else nc.scalar
                        eng.dma_start(
                            out=out_d[b_i, ct * 128:(ct + 1) * 128, :],
                            in_=ob)
            if debug and rep == reps - 1:
                nc.sync.dma_start(out=dbg_hv[:, :, :], in_=Hv)
                nc.sync.dma_start(out=dbg_hh[:, :, :], in_=Hh)
                nc.sync.dma_start(out=dbg_kt[:, :], in_=KT)

    nc.compile()
    return nc


def fcd_rearr(fcw_d):
    return fcw_d.rearrange("(kt p) n -> p kt n", kt=4)


_NC_CACHE = {}


def _get_nc(reps=1, debug=False, has_bias=True):
    key = (reps, debug, has_bias)
    if key not in _NC_CACHE:
        _NC_CACHE[key] = _build(reps=reps, debug=debug, has_bias=has_bias)
    return _NC_CACHE[key]


def _prep_core_inputs(x, weights_np):
    """Host-side marshalling for one core. x: [BL, C, H, W] f32."""
    bf = ml_dtypes.bfloat16
    m = {}
    m["xT"] = np.ascontiguousarray(
        x.transpose(1, 3, 0, 2).reshape(C, PLOC)).astype(bf)
    m["patchT"] = np.ascontiguousarray(
        x[:, :, ::3, ::3].reshape(BL, C, 100).transpose(0, 2, 1)).astype(bf)
    m.update(weights_np)
    return m


def _prep_weights(inputs):
    bf = ml_dtypes.bfloat16
    w = {}
    for L in _LSTMS:
        wih = np.asarray(inputs[L + "_Wih"], np.float32)
        whh = np.asarray(inputs[L + "_Whh"], np.float32)
        bih = np.asarray(inputs[L + "_bih"], np.float32)
        bhh = np.asarray(inputs[L + "_bhh"], np.float32)
        w[L + "_wih"] = np.ascontiguousarray(wih[_PERM].T).astype(bf)
        w[L + "_whh"] = np.ascontiguousarray(whh[_PERM].T).astype(bf)
        w[L + "_bias"] = np.ascontiguousarray(
            (bih + bhh)[_PERM].reshape(8, 128).T).astype(np.float32)
    w["fcw"] = np.asarray(inputs["fc_W"], np.float32).astype(bf)
    w["fcb"] = np.ascontiguousarray(
        np.asarray(inputs["fc_b"], np.float32).reshape(100, 1))
    return w


def run_cores(inputs, reps=1, debug=False):
    x = np.asarray(inputs["x"], np.float32)
    wnp = _prep_weights(inputs)
    has_bias = any(np.any(wnp[L + "_bias"]) for L in _LSTMS)
    nc = _get_nc(reps=reps, debug=debug, has_bias=has_bias)
    in_maps = [
        _prep_core_inputs(x[ci * BL:(ci + 1) * BL], wnp) for ci in range(N_CORES)
    ]
    res = run_bass_kernel_spmd(nc, in_maps, list(range(N_CORES)))
    return res


def kernel(**inputs) -> np.ndarray:
    res = run_cores(inputs)
    out = np.concatenate(
        [res.results[ci]["out"].reshape(BL, C, H, W) for ci in range(N_CORES)],
        axis=0)
    return out.astype(np.float32)



# revision 11
# speedup vs baseline: 1.4285x; 1.1035x over previous
"""PiCANet-G attention module as a Trainium2 Bass/Tile kernel.

Pure data-parallel over batch: 64 samples -> 8 cores x 8 samples.

Per core, three phases (SBUF-resident, fp8/bf16 matmuls, fp32 cell state):
  P1: vertical bi-LSTM over W (batch = 8*28 (b, h) rows, 28 steps, 2 dirs)
  P2: horizontal bi-LSTM over H (batch = 8*28 (b, w) rows)
  P3: fc -> softmax(100) -> per-sample einsum with the dilated 10x10 patch

Key structure:
  - Wih matmuls run in fp8-e4m3 DoubleRow mode (two 128-row k-tiles per
    instruction at double rate). Host-side scaling keeps fp8 values out of
    the subnormal range: x*4 (resp. h*8 for stage 2) and Wih*16 (resp. *8),
    so PSUM holds preactivations * 64; the gate activation applies
    scale=1/64. Whh stays bf16, scaled *64 on the host to match.
  - All four gates go through ONE fused Sigmoid activation: the g-gate's
    weight rows are additionally scaled *2 on the host and tanh(g) is
    recovered as 2*sigmoid(2g)-1 inside fused scalar_tensor_tensor ops.
  - Gate outputs land in SBUF as bf16; cell state stays fp32. h-writes run
    on the otherwise-idle GPSIMD engine.
  - P3 computes logits transposed ([100 taps, positions]), softmax over the
    partition dim via a ones-matmul + reciprocal + partition_broadcast, and
    the einsum contracts the 100 taps in a single k-tile.
"""

import numpy as np
import ml_dtypes
from contextlib import ExitStack

import concourse.bacc as bacc
import concourse.mybir as mybir
import concourse.tile as tile
from concourse.bass_utils import run_bass_kernel_spmd

# problem shapes (hardcoded per contract)
B, C, H, W = 64, 512, 28, 28
HID = 256
N_CORES = 8
BL = B // N_CORES        # samples per core
NB = BL * H              # 224 rows per LSTM step
T = 28                   # steps per LSTM
PLOC = BL * H * W        # 6272 positions per core

BF16 = mybir.dt.bfloat16
FP8 = mybir.dt.float8e4
F32 = mybir.dt.float32
AF = mybir.ActivationFunctionType
ALU = mybir.AluOpType
DR = mybir.MatmulPerfMode.DoubleRow

# torch gate order [i f g o] -> device order [i f o g] (g last; it gets the
# tanh-via-sigmoid treatment)
_PERM = np.concatenate([np.arange(0, 512), np.arange(768, 1024), np.arange(512, 768)])

_LSTMS = ["vf", "vb", "hf", "hb"]

# host-side scaling (see module docstring)
SC_PRE = 64.0      # preactivation scale absorbed by activation(scale=1/64)
SC_X = 4.0         # stage-1 fp8 input scale
SC_H8 = 8.0        # stage-2 fp8 input scale (h in (-1,1))


def _emit_lstm_step(nc, gpool, scr, wih8, whh_sb, bias_sb, ones224, x8rhs,
                    hprev, emit_h, c_fl, t, name, has_bias, pos):
    """One LSTM step for one direction.

    x8rhs(j, pos) -> [128, 2, 224] fp8 AP (k-tile pair j of the input)
    hprev(kk) -> [128, ...224] bf16 AP of h_{t-1}, or None at t == 0
    emit_h(So2, th, pos) emits the h-writes (So2/th are [128, 2, 224] views)
    c_fl: [128, 448] fp32 cell state
    """
    G = gpool.tile([128, 4, 512], F32, tag="G", name=f"G_{name}_{t}")
    for g in range(4):
        for h in range(2):
            m = g * 2 + h
            out_ap = G[:, g, h * 224:h * 224 + 224]
            for j in range(2):
                nc.tensor.matmul(
                    out_ap,
                    lhsT=wih8[:, 2 * j:2 * j + 2, m * 128:(m + 1) * 128],
                    rhs=x8rhs(j, pos),
                    start=(h == 0 and j == 0),
                    stop=(hprev is None and not has_bias and h == 1 and j == 1),
                    perf_mode=DR,
                )
            if has_bias:
                nc.tensor.matmul(
                    out_ap, lhsT=bias_sb[:, m * 128:(m + 1) * 128], rhs=ones224,
                    start=False, stop=(hprev is None and h == 1))
            if hprev is not None:
                for kk in range(2):
                    nc.tensor.matmul(
                        out_ap,
                        lhsT=whh_sb[:, kk, m * 128:(m + 1) * 128],
                        rhs=hprev(kk),
                        start=False, stop=(h == 1 and kk == 1))
    # fused activation: sigmoid for i,f,o and sigma(2g) for g (host scaled)
    S = scr.tile([128, 4, 448], BF16, tag="S", bufs=3, name=f"S_{name}_{t}")
    with nc.allow_low_precision(reason="lstm gates in bf16"):
        nc.scalar.activation(S, G[:, :, 0:448], AF.Sigmoid, scale=1.0 / SC_PRE)
    Si, Sf, So = S[:, 0, :], S[:, 1, :], S[:, 2, :]
    Sg = S[:, 3, :]
    # t1h = i * tanh(g) / 2 = (sigma(2g) - 0.5) * i
    t1h = scr.tile([128, 448], BF16, tag="t1h", bufs=3, name=f"t1_{name}_{t}")
    with nc.allow_low_precision(reason="i*tanh(g) in bf16"):
        nc.vector.scalar_tensor_tensor(t1h, Sg, -0.5, Si,
                                       op0=ALU.add, op1=ALU.mult)
    if t == 0:
        nc.vector.tensor_scalar_mul(c_fl, t1h, 2.0)
    else:
        nc.vector.tensor_mul(c_fl, Sf, c_fl)
        nc.vector.scalar_tensor_tensor(c_fl, t1h, 2.0, c_fl,
                                       op0=ALU.mult, op1=ALU.add)
    th = scr.tile([128, 448], BF16, tag="th", bufs=3, name=f"th_{name}_{t}")
    with nc.allow_low_precision(reason="tanh(c) in bf16"):
        nc.scalar.activation(th, c_fl, AF.Tanh)
    So2 = S.rearrange("p g (k r) -> p g k r", k=2)[:, 2]
    th2 = th.rearrange("p (k r) -> p k r", k=2)
    emit_h(So2, th2, pos)


def _build(reps=1, has_bias=True):
    nc = bacc.Bacc(None, target_bir_lowering=False)

    x8_d = nc.dram_tensor("x8", [C, PLOC], FP8, kind="ExternalInput")
    w_d = {}
    for L in _LSTMS:
        w_d[L + "_wih"] = nc.dram_tensor(L + "_wih", [512, 1024], FP8, kind="ExternalInput")
        w_d[L + "_whh"] = nc.dram_tensor(L + "_whh", [256, 1024], BF16, kind="ExternalInput")
        w_d[L + "_bias"] = nc.dram_tensor(L + "_bias", [1, 1024], BF16, kind="ExternalInput")
    fcw_d = nc.dram_tensor("fcw", [512, 100], BF16, kind="ExternalInput")
    fcb_d = nc.dram_tensor("fcb", [100, 1], F32, kind="ExternalInput")
    patchT_d = nc.dram_tensor("patchT", [BL, 100, 512], BF16, kind="ExternalInput")
    out_d = nc.dram_tensor("out", [BL, C, H * W], F32, kind="ExternalOutput")

    with tile.TileContext(nc) as tc, ExitStack() as ctx:
        wpool = ctx.enter_context(tc.tile_pool(name="wpool", bufs=1))
        bigA = ctx.enter_context(tc.tile_pool(name="bigA", bufs=1))
        bigB = ctx.enter_context(tc.tile_pool(name="bigB", bufs=1))
        state = ctx.enter_context(tc.tile_pool(name="state", bufs=1))
        scr = ctx.enter_context(tc.tile_pool(name="scr", bufs=3))

        # --- load weights; both stage-1 dirs first (step 0 needs them) ---
        wih_sb, whh_sb, bias_sb = {}, {}, {}
        for L in _LSTMS:
            wih_sb[L] = wpool.tile([128, 4, 1024], FP8, name=f"wih_{L}")
            whh_sb[L] = wpool.tile([128, 2, 1024], BF16, name=f"whh_{L}")
            bias_sb[L] = wpool.tile([1, 1024], BF16, name=f"bias_{L}")
        for L in ["vf", "vb"]:
            nc.sync.dma_start(out=wih_sb[L],
                              in_=w_d[L + "_wih"].rearrange("(kt p) m -> p kt m", kt=4))
        for L in ["vf", "vb"]:
            nc.scalar.dma_start(out=whh_sb[L],
                                in_=w_d[L + "_whh"].rearrange("(kt p) m -> p kt m", kt=2))
            if has_bias:
                nc.scalar.dma_start(out=bias_sb[L], in_=w_d[L + "_bias"][:, :])
        for L in ["hf", "hb"]:
            nc.sync.dma_start(out=wih_sb[L],
                              in_=w_d[L + "_wih"].rearrange("(kt p) m -> p kt m", kt=4))
            nc.sync.dma_start(out=whh_sb[L],
                              in_=w_d[L + "_whh"].rearrange("(kt p) m -> p kt m", kt=2))
            if has_bias:
                nc.sync.dma_start(out=bias_sb[L], in_=w_d[L + "_bias"][:, :])
        fcw_sb = wpool.tile([128, 4, 100], BF16, name="fcw_sb")
        nc.sync.dma_start(out=fcw_sb,
                          in_=fcw_d.rearrange("(kt p) n -> p kt n", kt=4))
        if has_bias:
            fcb_sb = wpool.tile([100, 1], F32, name="fcb_sb")
            nc.sync.dma_start(out=fcb_sb, in_=fcb_d[:, :])
        else:
            fcb_sb = None
        ones100 = wpool.tile([100, 1], BF16, name="ones100")
        nc.vector.memset(ones100, 1.0)
        ones224 = wpool.tile([1, 224], BF16, name="ones224")
        nc.vector.memset(ones224, 1.0)
        patchT_sb = wpool.tile([100, BL, 512], BF16, name="patchT_sb")
        nc.sync.dma_start(out=patchT_sb, in_=patchT_d.rearrange("b k c -> k b c"))

        for rep in range(reps):
            sfx = f"r{rep}"
            # --- P1: vertical bi-LSTM (steps over w; rows r = (b, h)) ---
            x8 = bigA.tile([128, 4, PLOC], FP8, tag="bigA", name=f"x8_{sfx}")
            xsrc = x8_d.rearrange("(kt p) f -> p kt f", kt=4)
            # stream in the order both directions consume: edges first
            wblocks = [(0, 3), (25, 28), (3, 8), (20, 25), (8, 14), (14, 20)]
            for lo, hi in wblocks:
                for kk in range(4):
                    nc.scalar.dma_start(out=x8[:, kk, lo * 224:hi * 224],
                                        in_=xsrc[:, kk, lo * 224:hi * 224])
            # Hv8: stage-1 h * 8 in fp8, laid out (h, b, w) for stage 2
            Hv8 = bigB.tile([128, 4, PLOC], FP8, tag="bigB", name=f"Hv8_{sfx}")
            Hv8v = Hv8.rearrange("p q (h b w) -> p q h b w", h=H, b=BL)

            with tc.tile_pool(name="gates1", bufs=2, space="PSUM") as gpool:
                cs = [state.tile([128, 448], F32, tag=f"c1_{d}",
                                 name=f"c1_{d}_{sfx}") for d in range(2)]
                hprev_t = [None, None]
                for t in range(T):
                    for d, L in enumerate(["vf", "vb"]):
                        pos = t if d == 0 else T - 1 - t

                        def x8rhs(j, p, _x8=x8):
                            return _x8[:, 2 * j:2 * j + 2, p * 224:(p + 1) * 224]

                        hp = hprev_t[d]
                        hprev = (None if hp is None
                                 else (lambda kk, _hp=hp: _hp[:, kk, :]))
                        hr = state.tile([128, 2, 224], BF16, tag=f"hr{d}",
                                        bufs=2, name=f"hr_{d}_{t}_{sfx}")

                        def emit_h(So2, th2, p, _hr=hr, _d=d):
                            # ring copy (bf16, rows (b, h)) for the recurrence
                            nc.gpsimd.tensor_mul(_hr, So2, th2)
                            # slab write (fp8, *8): dims (h, b) at fixed w;
                            # one op per k-half (walrus wants <=3D APs)
                            Sv = So2.rearrange("p k (b h) -> p k h b", b=BL)
                            tv = th2.rearrange("p k (b h) -> p k h b", b=BL)
                            with nc.allow_low_precision(reason="h*8 in fp8"):
                                for kk in range(2):
                                    nc.vector.scalar_tensor_tensor(
                                        Hv8v[:, 2 * _d + kk, :, :, p],
                                        Sv[:, kk], SC_H8, tv[:, kk],
                                        op0=ALU.mult, op1=ALU.mult)

                        _emit_lstm_step(nc, gpool, scr, wih_sb[L], whh_sb[L],
                                        bias_sb[L], ones224, x8rhs, hprev,
                                        emit_h, cs[d], t, f"1{L}{sfx}",
                                        has_bias, pos)
                        hprev_t[d] = hr

            # --- P2: horizontal bi-LSTM (steps over h; rows r = (b, w)) ---
            # Hh: bf16, laid out (b, h, w) for P3
            Hh = bigA.tile([128, 4, PLOC], BF16, tag="bigA", name=f"Hh_{sfx}")
            Hhv = Hh.rearrange("p q (b h w) -> p q b h w", b=BL, h=H)

            with tc.tile_pool(name="gates2", bufs=2, space="PSUM") as gpool:
                cs = [state.tile([128, 448], F32, tag=f"c2_{d}",
                                 name=f"c2_{d}_{sfx}") for d in range(2)]
                for t in range(T):
                    for d, L in enumerate(["hf", "hb"]):
                        pos = t if d == 0 else T - 1 - t
                        prev = pos - 1 if d == 0 else pos + 1

                        def x8rhs(j, p, _h=Hv8):
                            return _h[:, 2 * j:2 * j + 2, p * 224:(p + 1) * 224]

                        hprev = (None if t == 0 else
                                 (lambda kk, _d=d, _p=prev:
                                  Hhv[:, 2 * _d + kk, :, _p, :]))

                        def emit_h(So2, th2, p, _d=d):
                            Sv = So2.rearrange("p k (b w) -> p k b w", b=BL)
                            tv = th2.rearrange("p k (b w) -> p k b w", b=BL)
                            for kk in range(2):
                                nc.gpsimd.tensor_mul(
                                    Hhv[:, 2 * _d + kk, :, p, :],
                                    Sv[:, kk], tv[:, kk])

                        _emit_lstm_step(nc, gpool, scr, wih_sb[L], whh_sb[L],
                                        bias_sb[L], ones224, x8rhs, hprev,
                                        emit_h, cs[d], t, f"2{L}{sfx}",
                                        has_bias, pos)

            # --- P3: transposed fc (100 on partitions) + softmax + einsum ---
            # E[k, p] for positions p = (b, h, w); normalized in place.
            E = bigB.tile([100, PLOC], BF16, tag="bigB", name=f"E_{sfx}")
            NCH = 16
            CH = PLOC // NCH  # 392
            with tc.tile_pool(name="p3ps", bufs=2, space="PSUM") as pps:
                for c in range(NCH):
                    sl = slice(c * CH, (c + 1) * CH)
                    Lp = pps.tile([100, CH], F32, tag="L", name=f"L_{c}_{sfx}")
                    for kk in range(4):
                        nc.tensor.matmul(Lp, lhsT=fcw_sb[:, kk, :],
                                         rhs=Hh[:, kk, sl],
                                         start=(kk == 0), stop=(kk == 3))
                    # exp(L + fc_b): bias is per-partition in this layout
                    with nc.allow_low_precision(reason="softmax exp in bf16"):
                        if has_bias:
                            nc.scalar.activation(E[:, sl], Lp, AF.Exp, bias=fcb_sb)
                        else:
                            nc.scalar.activation(E[:, sl], Lp, AF.Exp)
                    Zp = pps.tile([1, CH], F32, tag="Z", name=f"Z_{c}_{sfx}")
                    nc.tensor.matmul(Zp, lhsT=ones100, rhs=E[:, sl],
                                     start=True, stop=True)
                    rz = scr.tile([1, CH], BF16, tag="rz", bufs=3,
                                  name=f"rz_{c}_{sfx}")
                    with nc.allow_low_precision(reason="softmax 1/Z in bf16"):
                        nc.vector.reciprocal(rz, Zp)
                    ib = scr.tile([100, CH], BF16, tag="ib", bufs=3,
                                  name=f"ib_{c}_{sfx}")
                    nc.gpsimd.partition_broadcast(ib, rz)
                    nc.vector.tensor_mul(E[:, sl], E[:, sl], ib)
                # einsum: out[c,(h,w)] = sum_k patch[c,k] En[k,(h,w)] per sample
                for b_i in range(BL):
                    for ct in range(4):
                        lhsT = patchT_sb[:, b_i, ct * 128:(ct + 1) * 128]
                        # [128, 1024] = 2 PSUM banks; each matmul output
                        # must stay inside one bank, so halves go at 0/512
                        Op = pps.tile([128, 2, 512], F32, tag="O", bufs=2,
                                      name=f"O_{b_i}_{ct}_{sfx}")
                        for j2 in range(2):
                            off = b_i * 784 + j2 * 392
                            nc.tensor.matmul(
                                Op[:, j2, 0:392], lhsT=lhsT,
                                rhs=E[:, off:off + 392],
                                start=True, stop=True)
                        ob = scr.tile([128, 2, 392], F32, tag="ob", bufs=3,
                                      name=f"ob_{b_i}_{ct}_{sfx}")
                        if ct % 2 == 0:
                            nc.vector.tensor_copy(ob, Op[:, :, 0:392])
                        else:
                            nc.scalar.copy(ob, Op[:, :, 0:392])
                        eng = nc.sync if ct % 2 == 0 else nc.scalar
                        eng.dma_start(
                            out=out_d[b_i, ct * 128:(ct + 1) * 128, :],
                            in_=ob)

    nc.compile()
    return nc


_NC_CACHE = {}


def _get_nc(reps=1, has_bias=True):
    key = (reps, has_bias)
    if key not in _NC_CACHE:
        _NC_CACHE[key] = _build(reps=reps, has_bias=has_bias)
    return _NC_CACHE[key]


def _prep_core_inputs(x, weights_np):
    """Host-side marshalling for one core. x: [BL, C, H, W] f32."""
    bf = ml_dtypes.bfloat16
    f8 = ml_dtypes.float8_e4m3
    m = {}
    m["x8"] = np.ascontiguousarray(
        (x * SC_X).transpose(1, 3, 0, 2).reshape(C, PLOC)).astype(f8)
    m["patchT"] = np.ascontiguousarray(
        x[:, :, ::3, ::3].reshape(BL, C, 100).transpose(0, 2, 1)).astype(bf)
    m.update(weights_np)
    return m


def _prep_weights(inputs):
    bf = ml_dtypes.bfloat16
    f8 = ml_dtypes.float8_e4m3
    # g-gate rows (after perm, 768:1024) get an extra *2: tanh via sigmoid
    gsc = np.ones((1024, 1), np.float32)
    gsc[768:] = 2.0
    w = {}
    for L in _LSTMS:
        wih = np.asarray(inputs[L + "_Wih"], np.float32)[_PERM] * gsc
        whh = np.asarray(inputs[L + "_Whh"], np.float32)[_PERM] * gsc
        bih = np.asarray(inputs[L + "_bih"], np.float32)
        bhh = np.asarray(inputs[L + "_bhh"], np.float32)
        sc_in = SC_PRE / (SC_X if L[0] == "v" else SC_H8)
        w[L + "_wih"] = np.ascontiguousarray((wih * sc_in).T).astype(f8)
        w[L + "_whh"] = np.ascontiguousarray((whh * SC_PRE).T).astype(bf)
        bias = ((bih + bhh)[_PERM].reshape(1024, 1) * gsc * SC_PRE)
        w[L + "_bias"] = np.ascontiguousarray(bias.reshape(1, 1024)).astype(bf)
    w["fcw"] = np.asarray(inputs["fc_W"], np.float32).astype(bf)
    w["fcb"] = np.ascontiguousarray(
        np.asarray(inputs["fc_b"], np.float32).reshape(100, 1))
    return w


def run_cores(inputs, reps=1):
    x = np.asarray(inputs["x"], np.float32)
    wnp = _prep_weights(inputs)
    has_bias = any(np.any(wnp[L + "_bias"]) for L in _LSTMS)
    nc = _get_nc(reps=reps, has_bias=has_bias)
    in_maps = [
        _prep_core_inputs(x[ci * BL:(ci + 1) * BL], wnp) for ci in range(N_CORES)
    ]
    res = run_bass_kernel_spmd(nc, in_maps, list(range(N_CORES)))
    return res


def kernel(**inputs) -> np.ndarray:
    res = run_cores(inputs)
    out = np.concatenate(
        [res.results[ci]["out"].reshape(BL, C, H, W) for ci in range(N_CORES)],
        axis=0)
    return out.astype(np.float32)


# revision 17
# speedup vs baseline: 1.5168x; 1.0618x over previous
"""PiCANet-G attention module as a Trainium2 Bass/Tile kernel.

Pure data-parallel over batch: 64 samples -> 8 cores x 8 samples.

Per core, three phases (SBUF-resident, fp8/bf16 matmuls, fp32 cell state):
  P1: vertical bi-LSTM over W (batch = 8*28 (b, h) rows, 28 steps, 2 dirs)
  P2: horizontal bi-LSTM over H (batch = 8*28 (b, w) rows)
  P3: fc -> softmax(100) -> per-sample einsum with the dilated 10x10 patch

Key structure:
  - Wih matmuls run in fp8-e4m3 DoubleRow mode (two 128-row k-tiles per
    instruction at double rate). Host-side scaling keeps fp8 values out of
    the subnormal range: x*4 (resp. h*8 for stage 2) and Wih*16 (resp. *8),
    so PSUM holds preactivations * 64; the gate activation applies
    scale=1/64. Whh stays bf16, scaled *64 on the host to match.
  - All four gates go through ONE fused Sigmoid activation: the g-gate's
    weight rows are additionally scaled *2 on the host and tanh(g) is
    recovered as 2*sigmoid(2g)-1 inside fused scalar_tensor_tensor ops.
  - Gate outputs land in SBUF as bf16; cell state stays fp32. h-writes run
    on the otherwise-idle GPSIMD engine.
  - P3 computes logits transposed ([100 taps, positions]), softmax over the
    partition dim via a ones-matmul + reciprocal + partition_broadcast, and
    the einsum contracts the 100 taps in a single k-tile.
"""

import numpy as np
import ml_dtypes
from contextlib import ExitStack

import concourse.bacc as bacc
import concourse.mybir as mybir
import concourse.tile as tile
from concourse.bass_utils import run_bass_kernel_spmd

# problem shapes (hardcoded per contract)
B, C, H, W = 64, 512, 28, 28
HID = 256
N_CORES = 8
BL = B // N_CORES        # samples per core
NB = BL * H              # 224 rows per LSTM step
T = 28                   # steps per LSTM
PLOC = BL * H * W        # 6272 positions per core

BF16 = mybir.dt.bfloat16
FP8 = mybir.dt.float8e4
F32 = mybir.dt.float32
AF = mybir.ActivationFunctionType
ALU = mybir.AluOpType
DR = mybir.MatmulPerfMode.DoubleRow

# torch gate order [i f g o] -> device order [i f o g] (g last; it gets the
# tanh-via-sigmoid treatment)
_PERM = np.concatenate([np.arange(0, 512), np.arange(768, 1024), np.arange(512, 768)])

_LSTMS = ["vf", "vb", "hf", "hb"]

# host-side scaling (see module docstring)
SC_PRE = 64.0      # preactivation scale absorbed by activation(scale=1/64)
SC_X = 4.0         # stage-1 fp8 input scale
SC_H8 = 8.0        # stage-2 fp8 input scale (h in (-1,1))


class _LstmStep:
    """One LSTM step for one direction, emitted in four pipeline stages so
    the two directions can interleave on each engine's queue.

    x8rhs(j, pos) -> [128, 2, 224] fp8 AP (k-tile pair j of the input)
    h8rhs(pos) -> [128, 2, 224] fp8 AP of h_{t-1}*8, or None -> bf16 path
      via hbrhs(kk, pos) -> [128, ..224] bf16 AP
    emit_h(So2, th, pos) emits the h-writes (So2/th are [128, 2, 224] views)
    c_fl: [128, 448] fp32 cell state
    """

    def __init__(self, nc, gpool, scr, wih8, whh_sb, bias_sb, ones224,
                 x8rhs, h8rhs, hbrhs, emit_h, c_fl, t, name, has_bias, pos):
        self.__dict__.update(locals())

    def mm(self):
        s, nc = self, self.nc
        first = s.t == 0
        self.G = s.gpool.tile([128, 4, 512], F32, tag="G",
                              name=f"G_{s.name}_{s.t}")
        for g in range(4):
            for h in range(2):
                m = g * 2 + h
                out_ap = self.G[:, g, h * 224:h * 224 + 224]
                for j in range(2):
                    nc.tensor.matmul(
                        out_ap,
                        lhsT=s.wih8[:, 2 * j:2 * j + 2, m * 128:(m + 1) * 128],
                        rhs=s.x8rhs(j, s.pos),
                        start=(h == 0 and j == 0),
                        stop=(first and not s.has_bias and h == 1 and j == 1),
                        perf_mode=DR,
                    )
                if s.has_bias:
                    nc.tensor.matmul(
                        out_ap, lhsT=s.bias_sb[:, m * 128:(m + 1) * 128],
                        rhs=s.ones224, start=False, stop=(first and h == 1))
        if not first:
            for g in range(4):
                for h in range(2):
                    m = g * 2 + h
                    out_ap = self.G[:, g, h * 224:h * 224 + 224]
                    if s.h8rhs is not None:
                        nc.tensor.matmul(
                            out_ap,
                            lhsT=s.whh_sb[:, 0:2, m * 128:(m + 1) * 128],
                            rhs=s.h8rhs(s.pos),
                            start=False, stop=(h == 1), perf_mode=DR)
                    else:
                        for kk in range(2):
                            nc.tensor.matmul(
                                out_ap,
                                lhsT=s.whh_sb[:, kk, m * 128:(m + 1) * 128],
                                rhs=s.hbrhs(kk, s.pos),
                                start=False, stop=(h == 1 and kk == 1))

    def sigma(self):
        s, nc = self, self.nc
        # fused: sigmoid for i,f,o and sigma(2g) for g (host-scaled rows)
        self.S = s.scr.tile([128, 4, 448], BF16, tag="S", bufs=4,
                            name=f"S_{s.name}_{s.t}")
        with nc.allow_low_precision(reason="lstm gates in bf16"):
            nc.scalar.activation(self.S, self.G[:, :, 0:448], AF.Sigmoid,
                                 scale=1.0 / SC_PRE)

    def cupd(self):
        s, nc = self, self.nc
        S, c_fl = self.S, s.c_fl
        Si, Sf, Sg = S[:, 0, :], S[:, 1, :], S[:, 3, :]
        # t1h = i * tanh(g) / 2 = (sigma(2g) - 0.5) * i
        t1h = s.scr.tile([128, 448], BF16, tag="t1h", bufs=3,
                         name=f"t1_{s.name}_{s.t}")
        with nc.allow_low_precision(reason="i*tanh(g) in bf16"):
            nc.vector.scalar_tensor_tensor(t1h, Sg, -0.5, Si,
                                           op0=ALU.add, op1=ALU.mult)
        if s.t == 0:
            nc.vector.tensor_scalar_mul(c_fl, t1h, 2.0)
        else:
            nc.vector.tensor_mul(c_fl, Sf, c_fl)
            nc.vector.scalar_tensor_tensor(c_fl, t1h, 2.0, c_fl,
                                           op0=ALU.mult, op1=ALU.add)

    def tail(self):
        s, nc = self, self.nc
        th = s.scr.tile([128, 448], BF16, tag="th", bufs=3,
                        name=f"th_{s.name}_{s.t}")
        with nc.allow_low_precision(reason="tanh(c) in bf16"):
            nc.scalar.activation(th, s.c_fl, AF.Tanh)
        So2 = self.S.rearrange("p g (k r) -> p g k r", k=2)[:, 2]
        th2 = th.rearrange("p (k r) -> p k r", k=2)
        s.emit_h(So2, th2, s.pos)


def _build(reps=1, has_bias=True):
    nc = bacc.Bacc(None, target_bir_lowering=False)

    x8_d = nc.dram_tensor("x8", [C, PLOC], FP8, kind="ExternalInput")
    w_d = {}
    for L in _LSTMS:
        whh_dt = FP8 if L[0] == "v" else BF16
        w_d[L + "_wih"] = nc.dram_tensor(L + "_wih", [512, 1024], FP8, kind="ExternalInput")
        w_d[L + "_whh"] = nc.dram_tensor(L + "_whh", [256, 1024], whh_dt, kind="ExternalInput")
        w_d[L + "_bias"] = nc.dram_tensor(L + "_bias", [1, 1024], BF16, kind="ExternalInput")
    fcw_d = nc.dram_tensor("fcw", [512, 100], BF16, kind="ExternalInput")
    fcb_d = nc.dram_tensor("fcb", [100, 1], F32, kind="ExternalInput")
    patchT_d = nc.dram_tensor("patchT", [BL, 100, 512], BF16, kind="ExternalInput")
    out_d = nc.dram_tensor("out", [BL, C, H * W], F32, kind="ExternalOutput")

    with tile.TileContext(nc) as tc, ExitStack() as ctx:
        wpool = ctx.enter_context(tc.tile_pool(name="wpool", bufs=1))
        bigA = ctx.enter_context(tc.tile_pool(name="bigA", bufs=1))
        bigB = ctx.enter_context(tc.tile_pool(name="bigB", bufs=1))
        state = ctx.enter_context(tc.tile_pool(name="state", bufs=1))
        scr = ctx.enter_context(tc.tile_pool(name="scr", bufs=3))

        # --- load weights; both stage-1 dirs first (step 0 needs them) ---
        wih_sb, whh_sb, bias_sb = {}, {}, {}
        for L in _LSTMS:
            whh_dt = FP8 if L[0] == "v" else BF16
            wih_sb[L] = wpool.tile([128, 4, 1024], FP8, name=f"wih_{L}")
            whh_sb[L] = wpool.tile([128, 2, 1024], whh_dt, name=f"whh_{L}")
            bias_sb[L] = wpool.tile([1, 1024], BF16, name=f"bias_{L}")
        for L in ["vf", "vb"]:
            nc.sync.dma_start(out=wih_sb[L],
                              in_=w_d[L + "_wih"].rearrange("(kt p) m -> p kt m", kt=4))
        for L in ["vf", "vb"]:
            nc.scalar.dma_start(out=whh_sb[L],
                                in_=w_d[L + "_whh"].rearrange("(kt p) m -> p kt m", kt=2))
            if has_bias:
                nc.scalar.dma_start(out=bias_sb[L], in_=w_d[L + "_bias"][:, :])
        for L in ["hf", "hb"]:
            nc.sync.dma_start(out=wih_sb[L],
                              in_=w_d[L + "_wih"].rearrange("(kt p) m -> p kt m", kt=4))
            nc.sync.dma_start(out=whh_sb[L],
                              in_=w_d[L + "_whh"].rearrange("(kt p) m -> p kt m", kt=2))
            if has_bias:
                nc.sync.dma_start(out=bias_sb[L], in_=w_d[L + "_bias"][:, :])
        fcw_sb = wpool.tile([128, 4, 100], BF16, name="fcw_sb")
        nc.sync.dma_start(out=fcw_sb,
                          in_=fcw_d.rearrange("(kt p) n -> p kt n", kt=4))
        if has_bias:
            fcb_sb = wpool.tile([100, 1], F32, name="fcb_sb")
            nc.sync.dma_start(out=fcb_sb, in_=fcb_d[:, :])
        else:
            fcb_sb = None
        ones100 = wpool.tile([100, 1], BF16, name="ones100")
        nc.vector.memset(ones100, 1.0)
        ones224 = wpool.tile([1, 224], BF16, name="ones224")
        nc.vector.memset(ones224, 1.0)
        patchT_sb = wpool.tile([100, BL, 512], BF16, name="patchT_sb")
        nc.sync.dma_start(out=patchT_sb, in_=patchT_d.rearrange("b k c -> k b c"))

        for rep in range(reps):
            sfx = f"r{rep}"
            # --- P1: vertical bi-LSTM (steps over w; rows r = (h, b)) ---
            x8 = bigA.tile([128, 4, PLOC], FP8, tag="bigA", name=f"x8_{sfx}")
            xsrc = x8_d.rearrange("(kt p) f -> p kt f", kt=4)
            # stream in the order both directions consume: edges first
            wblocks = [(0, 3), (25, 28), (3, 8), (20, 25), (8, 14), (14, 20)]
            for lo, hi in wblocks:
                for kk in range(4):
                    nc.scalar.dma_start(out=x8[:, kk, lo * 224:hi * 224],
                                        in_=xsrc[:, kk, lo * 224:hi * 224])
            # Hv8: stage-1 h * 8 in fp8, laid out (h, b, w) for stage 2.
            # With stage-1 rows r = (h, b), the slab at fixed w is the
            # stride-28 view [p, q, w, r].
            Hv8 = bigB.tile([128, 4, PLOC], FP8, tag="bigB", name=f"Hv8_{sfx}")
            Hv8f = Hv8.rearrange("p q (f w) -> p q w f", w=W)

            with tc.tile_pool(name="gates1", bufs=2, space="PSUM") as gpool:
                cs = [state.tile([128, 448], F32, tag=f"c1_{d}",
                                 name=f"c1_{d}_{sfx}") for d in range(2)]

                def mk_step1(t, d, L):
                    pos = t if d == 0 else T - 1 - t
                    prev = pos - 1 if d == 0 else pos + 1

                    def x8rhs(j, p, _x8=x8):
                        return _x8[:, 2 * j:2 * j + 2, p * 224:(p + 1) * 224]

                    h8rhs = (None if t == 0 else
                             (lambda p, _d=d, _pv=prev:
                              Hv8f[:, 2 * _d:2 * _d + 2, _pv, :]))

                    def emit_h(So2, th2, p, _d=d):
                        # single fp8 slab write (h * 8), stride-28 dst
                        with nc.allow_low_precision(reason="h*8 in fp8"):
                            nc.vector.scalar_tensor_tensor(
                                Hv8f[:, 2 * _d:2 * _d + 2, p, :],
                                So2, SC_H8, th2,
                                op0=ALU.mult, op1=ALU.mult)

                    return _LstmStep(nc, gpool, scr, wih_sb[L], whh_sb[L],
                                     bias_sb[L], ones224, x8rhs, h8rhs, None,
                                     emit_h, cs[d], t, f"1{L}{sfx}",
                                     has_bias, pos)

                for t in range(T):
                    steps = [mk_step1(t, d, L)
                             for d, L in enumerate(["vf", "vb"])]
                    for st in steps:
                        st.mm()
                    for st in steps:
                        st.sigma()
                    for st in steps:
                        st.cupd()
                    for st in steps:
                        st.tail()

            # --- P2: horizontal bi-LSTM (steps over h; rows r = (b, w)) ---
            # Hh: bf16, laid out (b, h, w) for P3
            Hh = bigA.tile([128, 4, PLOC], BF16, tag="bigA", name=f"Hh_{sfx}")
            Hhv = Hh.rearrange("p q (b h w) -> p q b h w", b=BL, h=H)

            with tc.tile_pool(name="gates2", bufs=2, space="PSUM") as gpool:
                cs = [state.tile([128, 448], F32, tag=f"c2_{d}",
                                 name=f"c2_{d}_{sfx}") for d in range(2)]

                def mk_step2(t, d, L):
                    pos = t if d == 0 else T - 1 - t
                    prev = pos - 1 if d == 0 else pos + 1

                    def x8rhs(j, p, _h=Hv8):
                        return _h[:, 2 * j:2 * j + 2, p * 224:(p + 1) * 224]

                    hbrhs = (None if t == 0 else
                             (lambda kk, p, _d=d, _pv=prev:
                              Hhv[:, 2 * _d + kk, :, _pv, :]))

                    def emit_h(So2, th2, p, _d=d):
                        Sv = So2.rearrange("p k (b w) -> p k b w", b=BL)
                        tv = th2.rearrange("p k (b w) -> p k b w", b=BL)
                        for kk in range(2):
                            nc.gpsimd.tensor_mul(
                                Hhv[:, 2 * _d + kk, :, p, :],
                                Sv[:, kk], tv[:, kk])

                    return _LstmStep(nc, gpool, scr, wih_sb[L], whh_sb[L],
                                     bias_sb[L], ones224, x8rhs, None, hbrhs,
                                     emit_h, cs[d], t, f"2{L}{sfx}",
                                     has_bias, pos)

                for t in range(T):
                    steps = [mk_step2(t, d, L)
                             for d, L in enumerate(["hf", "hb"])]
                    for st in steps:
                        st.mm()
                    for st in steps:
                        st.sigma()
                    for st in steps:
                        st.cupd()
                    for st in steps:
                        st.tail()

            # --- P3: transposed fc (100 on partitions) + softmax + einsum ---
            # E[k, p] for positions p = (b, h, w); normalized in place.
            E = bigB.tile([100, PLOC], BF16, tag="bigB", name=f"E_{sfx}")
            NCH = 16
            CH = PLOC // NCH  # 392
            with tc.tile_pool(name="p3ps", bufs=2, space="PSUM") as pps:
                for c in range(NCH):
                    sl = slice(c * CH, (c + 1) * CH)
                    Lp = pps.tile([100, CH], F32, tag="L", name=f"L_{c}_{sfx}")
                    for kk in range(4):
                        nc.tensor.matmul(Lp, lhsT=fcw_sb[:, kk, :],
                                         rhs=Hh[:, kk, sl],
                                         start=(kk == 0), stop=(kk == 3))
                    # exp(L + fc_b): bias is per-partition in this layout
                    with nc.allow_low_precision(reason="softmax exp in bf16"):
                        if has_bias:
                            nc.scalar.activation(E[:, sl], Lp, AF.Exp, bias=fcb_sb)
                        else:
                            nc.scalar.activation(E[:, sl], Lp, AF.Exp)
                    Zp = pps.tile([1, CH], F32, tag="Z", name=f"Z_{c}_{sfx}")
                    nc.tensor.matmul(Zp, lhsT=ones100, rhs=E[:, sl],
                                     start=True, stop=True)
                    rz = scr.tile([1, CH], BF16, tag="rz", bufs=3,
                                  name=f"rz_{c}_{sfx}")
                    with nc.allow_low_precision(reason="softmax 1/Z in bf16"):
                        nc.vector.reciprocal(rz, Zp)
                    ib = scr.tile([100, CH], BF16, tag="ib", bufs=3,
                                  name=f"ib_{c}_{sfx}")
                    nc.gpsimd.partition_broadcast(ib, rz)
                    nc.vector.tensor_mul(E[:, sl], E[:, sl], ib)
                # einsum: out[c,(h,w)] = sum_k patch[c,k] En[k,(h,w)] per sample
                for b_i in range(BL):
                    for ct in range(4):
                        lhsT = patchT_sb[:, b_i, ct * 128:(ct + 1) * 128]
                        # [128, 1024] = 2 PSUM banks; each matmul output
                        # must stay inside one bank, so halves go at 0/512
                        Op = pps.tile([128, 2, 512], F32, tag="O", bufs=2,
                                      name=f"O_{b_i}_{ct}_{sfx}")
                        for j2 in range(2):
                            off = b_i * 784 + j2 * 392
                            nc.tensor.matmul(
                                Op[:, j2, 0:392], lhsT=lhsT,
                                rhs=E[:, off:off + 392],
                                start=True, stop=True)
                        ob = scr.tile([128, 2, 392], F32, tag="ob", bufs=3,
                                      name=f"ob_{b_i}_{ct}_{sfx}")
                        if ct % 2 == 0:
                            nc.vector.tensor_copy(ob, Op[:, :, 0:392])
                        else:
                            nc.scalar.copy(ob, Op[:, :, 0:392])
                        eng = nc.sync if ct % 2 == 0 else nc.scalar
                        eng.dma_start(
                            out=out_d[b_i, ct * 128:(ct + 1) * 128, :],
                            in_=ob)

    nc.compile()
    return nc


_NC_CACHE = {}


def _get_nc(reps=1, has_bias=True):
    key = (reps, has_bias)
    if key not in _NC_CACHE:
        _NC_CACHE[key] = _build(reps=reps, has_bias=has_bias)
    return _NC_CACHE[key]


def _prep_core_inputs(x, weights_np):
    """Host-side marshalling for one core. x: [BL, C, H, W] f32."""
    bf = ml_dtypes.bfloat16
    f8 = ml_dtypes.float8_e4m3
    m = {}
    # stage-1 rows r = (h, b): layout (C, W, H, B)
    m["x8"] = np.ascontiguousarray(
        (x * SC_X).transpose(1, 3, 2, 0).reshape(C, PLOC)).astype(f8)
    m["patchT"] = np.ascontiguousarray(
        x[:, :, ::3, ::3].reshape(BL, C, 100).transpose(0, 2, 1)).astype(bf)
    m.update(weights_np)
    return m


def _prep_weights(inputs):
    bf = ml_dtypes.bfloat16
    f8 = ml_dtypes.float8_e4m3
    # g-gate rows (after perm, 768:1024) get an extra *2: tanh via sigmoid
    gsc = np.ones((1024, 1), np.float32)
    gsc[768:] = 2.0
    w = {}
    for L in _LSTMS:
        wih = np.asarray(inputs[L + "_Wih"], np.float32)[_PERM] * gsc
        whh = np.asarray(inputs[L + "_Whh"], np.float32)[_PERM] * gsc
        bih = np.asarray(inputs[L + "_bih"], np.float32)
        bhh = np.asarray(inputs[L + "_bhh"], np.float32)
        sc_in = SC_PRE / (SC_X if L[0] == "v" else SC_H8)
        w[L + "_wih"] = np.ascontiguousarray((wih * sc_in).T).astype(f8)
        if L[0] == "v":
            # stage-1 Whh runs fp8 DoubleRow against the h*8 slab
            w[L + "_whh"] = np.ascontiguousarray(
                (whh * (SC_PRE / SC_H8)).T).astype(f8)
        else:
            w[L + "_whh"] = np.ascontiguousarray((whh * SC_PRE).T).astype(bf)
        bias = ((bih + bhh)[_PERM].reshape(1024, 1) * gsc * SC_PRE)
        w[L + "_bias"] = np.ascontiguousarray(bias.reshape(1, 1024)).astype(bf)
    w["fcw"] = np.asarray(inputs["fc_W"], np.float32).astype(bf)
    w["fcb"] = np.ascontiguousarray(
        np.asarray(inputs["fc_b"], np.float32).reshape(100, 1))
    return w


def run_cores(inputs, reps=1):
    x = np.asarray(inputs["x"], np.float32)
    wnp = _prep_weights(inputs)
    has_bias = any(np.any(wnp[L + "_bias"]) for L in _LSTMS)
    nc = _get_nc(reps=reps, has_bias=has_bias)
    in_maps = [
        _prep_core_inputs(x[ci * BL:(ci + 1) * BL], wnp) for ci in range(N_CORES)
    ]
    res = run_bass_kernel_spmd(nc, in_maps, list(range(N_CORES)))
    return res


def kernel(**inputs) -> np.ndarray:
    res = run_cores(inputs)
    out = np.concatenate(
        [res.results[ci]["out"].reshape(BL, C, H, W) for ci in range(N_CORES)],
        axis=0)
    return out.astype(np.float32)
